# revision 1
# baseline (speedup 1.0000x reference)
# Multi-head causal attention (B=4, S=2048, D=1024, H=16) on 8 NeuronCores.
#
# Sharding: core c handles batch b = c//2 and head-group hg = c%2 (8 heads,
# 512 of the 1024 qkv dims). Every core runs an identical program (SPMD);
# only its input data differs. Per-core work:
#   - Q/K/V projections for its 512 columns (contract din via transposed x)
#   - causal attention for its 8 heads over the full sequence
#   - partial out-projection ctx_local @ Wo[rows of its heads]
# The two cores sharing a batch produce partial outputs that are summed on
# the host (out_proj tensor-parallel reduction). bo is added on hg==0 cores
# only (hg==1 cores receive zeros) so the host-side combine is a plain add.
#
# The program is emitted interleaved: projections for s-tile st, then
# attention for q-tile st (whose keys stop at st by causality), then the
# out-projection for that q-tile. The Tile scheduler's per-engine priority
# heap then fills the PE idle slots that arise while ACT computes exp()
# with projection/out-projection matmuls from the next tile, keeping both
# engines saturated.
#
# Attention per (q-tile of 512, head pair hp, kpos chunk kc of 128):
#   scoresT[kpos, q] for both heads of the pair land in one 2-bank PSUM
#   tile [128, 1024]; heads are packed into PE array rows 0-63 / 64-127
#   (row-tiled, so the two matmuls overlap on HW). exp() runs fused over
#   both heads on ACT -> bf16; diagonal chunks are column-sliced to the
#   causally valid region and masked with a materialized [128, 2*128] mask
#   (dense operands keep DVE in its 2x perf mode). The AV matmul uses
#   lhsT=[V_h | 1] so PSUM row 64 accumulates the softmax denominator.
#   Normalization: one DVE reciprocal over both heads' denominators, then
#   a GPSIMD partition_broadcast replicates 1/denom across the 64 head
#   rows (frees PE + DVE from the broadcast), then one DVE multiply.

import numpy as np
import ml_dtypes

import concourse.bass as bass
import concourse.mybir as mybir
import concourse.tile as tile
from concourse import bass_utils

B, S, D = 4, 2048, 1024
H, HD = 16, 64
HG = 2            # tensor-parallel head groups
HL = H // HG      # 8 local heads
DL = HL * HD      # 512 local qkv dims
P = 128
QT_W = 512        # q tile width in attention
NQT = S // QT_W   # 4
NKC = S // P      # 16 kpos chunks
NDC = D // P      # 8 din chunks
NDT = DL // P     # 4 dout tiles (head pairs)
F32 = mybir.dt.float32
BF16 = mybir.dt.bfloat16
# bf16 bit-trick exp on DVE for alternate qt=3 full chunks. Disabled: CoreSim
# shows the DVE op (~1.7us vs ACT's ~0.9us) lengthens the chunk pipeline's
# latency-bound sections more than it relieves ACT.
DVE_EXP = False

_BUILT = {}


def _split_waits(nc):
    """The walrus build in this env encodes at most 1 sync wait per
    instruction (2 for EventSemaphore) and refuses instructions with more.
    Move excess waits onto preceding same-engine NoOps."""
    n_new = 0
    for f in nc.m.functions:
        for bb in f.blocks:
            insts = bb.instructions
            out = []
            changed = False
            for ins in insts:
                si = ins.sync_info
                cap = 2 if ins.opcode == "EventSemaphore" else 1
                if si is not None and si.on_wait and len(si.on_wait) > cap:
                    waits = list(si.on_wait)
                    for k, w in enumerate(waits[:-cap]):
                        nop = mybir.InstNoOp(name=f"{ins.name}_sw{k}")
                        nop.engine = ins.engine
                        nop.sync_info = mybir.SyncInfo(on_wait=[w], on_update=[])
                        out.append(nop)
                        n_new += 1
                    ins.sync_info = mybir.SyncInfo(
                        on_wait=waits[-cap:], on_update=list(si.on_update)
                    )
                    changed = True
                out.append(ins)
            if changed:
                insts[:] = out
                assert len(bb.instructions) == len(out)
    return n_new


def _build_kernel(reps=1, parts="full"):
    nc = bass.Bass(
        "TRN2",
        target_bir_lowering=False,
        debug=False,
        enable_asserts=False,
        num_devices=8,
    )

    x_d = nc.dram_tensor("x", [S, D], BF16, kind="ExternalInput").ap()
    wq_d = nc.dram_tensor("wq", [D, DL], BF16, kind="ExternalInput").ap()
    wk_d = nc.dram_tensor("wk", [D, DL], BF16, kind="ExternalInput").ap()
    wv_d = nc.dram_tensor("wv", [D, DL], BF16, kind="ExternalInput").ap()
    wo_d = nc.dram_tensor("wo", [DL, D], BF16, kind="ExternalInput").ap()
    bq_d = nc.dram_tensor("bq", [P, NDT], F32, kind="ExternalInput").ap()
    bk_d = nc.dram_tensor("bk", [P, NDT], F32, kind="ExternalInput").ap()
    bv_d = nc.dram_tensor("bv", [P, DL], F32, kind="ExternalInput").ap()
    bo_d = nc.dram_tensor("bo", [P, D], F32, kind="ExternalInput").ap()
    mask_d = nc.dram_tensor("mask", [P, 2 * P], BF16, kind="ExternalInput").ap()
    out_d = nc.dram_tensor("out", [S, D], F32, kind="ExternalOutput").ap()

    with tile.TileContext(nc) as tc:
        with (
            tc.tile_pool(name="const", bufs=1) as const,
            tc.tile_pool(name="resid", bufs=1) as resid,
            tc.tile_pool(name="expp", bufs=6) as expp,
            tc.tile_pool(name="npool", bufs=3) as npool,
            tc.tile_pool(name="osb", bufs=3) as opool,
            tc.tile_pool(name="ppsum", bufs=2, space="PSUM") as ppsum,
            tc.tile_pool(name="spsum", bufs=2, space="PSUM") as spsum,
            tc.tile_pool(name="cpsum", bufs=1, space="PSUM") as cpsum,
        ):
            # ---- constants ----
            bq_sb = const.tile([P, NDT], F32)
            nc.sync.dma_start(bq_sb[:], bq_d)
            bk_sb = const.tile([P, NDT], F32)
            nc.sync.dma_start(bk_sb[:], bk_d)
            bv_bc = const.tile([P, DL], F32)
            nc.sync.dma_start(bv_bc[:], bv_d)
            bo_bc = const.tile([P, D], F32)
            nc.sync.dma_start(bo_bc[:], bo_d)
            mask_sb = const.tile([P, 2, P], BF16)
            nc.sync.dma_start(mask_sb[:], mask_d.rearrange("p (h q) -> p h q", h=2))
            ones_hd = const.tile([1, HD], BF16)
            nc.vector.memset(ones_hd[:], 1.0)
            warm_rhs = const.tile([1, P], BF16)
            nc.vector.memset(warm_rhs[:], 1.0)

            # PE warm-up: the HAM clock gate holds the PE at 1.2 GHz until it
            # has been busy ~3.4us, and the initial weight/x DMAs leave the
            # PE idle about that long. Burn the DMA wait on dummy matmuls so
            # the first projection runs at the full 2.4 GHz.
            for _w in range(155):
                pw = ppsum.tile([P, QT_W], F32, tag="pp", name="warm")
                nc.tensor.matmul(
                    pw[0:HD, 0:P], ones_hd[:], warm_rhs[:],
                    start=True, stop=True,
                )

            for _rep in range(reps):
                # ---- DMA prefetch: weights + transposed x, in the order the
                # interleaved compute consumes them ----
                wq_t, wk_t, wv_t, wo_t = {}, {}, {}, {}

                def _load_w(pre, w_d, w_t, nch, wid):
                    for dc in range(nch):
                        t = resid.tile(
                            [P, wid], BF16, name=f"{pre}{dc}_r{_rep}", tag=f"{pre}{dc}"
                        )
                        nc.sync.dma_start(t[:], w_d[dc * P : (dc + 1) * P, :])
                        w_t[dc] = t

                xT = {}

                def _load_xt(st):
                    for dc in range(NDC):
                        t = resid.tile(
                            [P, QT_W], BF16, name=f"xT_{st}_{dc}_r{_rep}",
                            tag=f"xT_{st}_{dc}",
                        )
                        nc.sync.dma_start_transpose(
                            t[:],
                            x_d[st * QT_W : (st + 1) * QT_W, dc * P : (dc + 1) * P],
                        )
                        xT[st, dc] = t

                _load_w("wv", wv_d, wv_t, NDC, DL)
                _load_xt(0)
                _load_w("wq", wq_d, wq_t, NDC, DL)
                _load_w("wk", wk_d, wk_t, NDC, DL)
                _load_xt(1)
                _load_w("wo", wo_d, wo_t, NDT, D)
                _load_xt(2)
                _load_xt(3)

                v_t = {}

                def _proj_v(sc):
                    # V projection for kpos chunk sc (natural layout + ones col)
                    pv = ppsum.tile([P, DL], F32, tag="pp", name=f"pv_r{_rep}")
                    for dc in range(NDC):
                        nc.tensor.matmul(
                            pv[:],
                            xT[sc // 4, dc][:, (sc % 4) * P : (sc % 4 + 1) * P],
                            wv_t[dc][:],
                            start=(dc == 0),
                            stop=(dc == NDC - 1),
                        )
                    vtag = f"v{sc}" + (f"_p{_rep % 2}" if sc < 4 else "")
                    vt = resid.tile(
                        [P, HL, HD + 1], BF16, name=f"v{sc}_r{_rep}", tag=vtag
                    )
                    nc.vector.tensor_tensor(
                        vt[:, :, 0:HD],
                        pv[:].rearrange("p (h e) -> p h e", e=HD),
                        bv_bc[:].rearrange("p (h e) -> p h e", e=HD),
                        mybir.AluOpType.add,
                    )
                    nc.vector.memset(vt[:, :, HD : HD + 1], 1.0)
                    v_t[sc] = vt

                qT, kT = {}, {}

                def _proj_qk(st):
                    # Q/K projections (transposed layout, per (dt, st) tiles)
                    for dt in range(NDT):
                        pq = ppsum.tile([P, QT_W], F32, tag="pp", name=f"pq_r{_rep}")
                        for dc in range(NDC):
                            nc.tensor.matmul(
                                pq[:],
                                wq_t[dc][:, dt * P : (dt + 1) * P],
                                xT[st, dc][:],
                                start=(dc == 0),
                                stop=(dc == NDC - 1),
                            )
                        qpar = f"_p{_rep % 2}" if st == 0 else ""
                        qt_t = resid.tile(
                            [P, QT_W], BF16, name=f"qT{dt}_{st}_r{_rep}",
                            tag=f"qT{dt}_{st}" + qpar,
                        )
                        nc.vector.tensor_scalar(
                            qt_t[:], pq[:], bq_sb[:, dt : dt + 1], 0.125,
                            mybir.AluOpType.add, mybir.AluOpType.mult,
                        )
                        qT[dt, st] = qt_t

                        pk = ppsum.tile([P, QT_W], F32, tag="pp", name=f"pk_r{_rep}")
                        for dc in range(NDC):
                            nc.tensor.matmul(
                                pk[:],
                                wk_t[dc][:, dt * P : (dt + 1) * P],
                                xT[st, dc][:],
                                start=(dc == 0),
                                stop=(dc == NDC - 1),
                            )
                        kt_t = resid.tile(
                            [P, QT_W], BF16, name=f"kT{dt}_{st}_r{_rep}",
                            tag=f"kT{dt}_{st}" + qpar,
                        )
                        nc.vector.tensor_scalar(
                            kt_t[:], pk[:], bk_sb[:, dt : dt + 1], None,
                            mybir.AluOpType.add,
                        )
                        kT[dt, st] = kt_t

                ctxt = {}

                def _attn(qt, pending):
                    # attention for q tile qt over all head pairs
                    nk = (qt + 1) * (QT_W // P)
                    for hp in range(NDT):
                        cpar = f"_p{_rep % 2}" if qt == 0 else ""
                        ct = resid.tile(
                            [P, QT_W], BF16, name=f"ctx{hp}_{qt}_r{_rep}",
                            tag=f"ctx{hp}_{qt}" + cpar,
                        )
                        ctxt[hp, qt] = ct
                        # both heads' ctx in one tile: head A cols 0:512,
                        # head B cols 512:1024; row 64 = softmax denominators
                        pc = cpsum.tile([P, 2 * QT_W], F32, tag="ctx", name=f"pc_r{_rep}")
                        for kc in range(nk):
                            dj = kc - qt * (QT_W // P)   # >=0 on diagonal chunks
                            off = max(0, dj) * P
                            ps = spsum.tile([P, 2 * QT_W], F32, tag="sc", name=f"ps_r{_rep}")
                            for hh in range(2):
                                poff = hh * HD
                                nc.tensor.matmul(
                                    ps[:, hh * QT_W + off : (hh + 1) * QT_W],
                                    kT[hp, kc // 4][
                                        poff : poff + HD, (kc % 4) * P : (kc % 4 + 1) * P
                                    ],
                                    qT[hp, qt][poff : poff + HD, off:QT_W],
                                    start=True,
                                    stop=True,
                                )
                            # In the last q tile no projection work remains to
                            # fill PE bubbles, so ACT's exp throughput paces
                            # the pipeline. Offload alternate full chunks to
                            # DVE with a bf16 Schraudolph bit-trick exp:
                            #   bf16(x) ~= bitcast16(round(x*128/ln2 + 16248.5))
                            # (~1.7% weight error on those chunks; softmax
                            # renormalization keeps the net output error well
                            # inside the tolerance).
                            dve_exp = DVE_EXP and dj < 0 and qt == NQT - 1 and kc % 2 == 0
                            if dve_exp:
                                es_i = expp.tile(
                                    [P, 2, QT_W], mybir.dt.int16, tag="exp", name=f"esi_r{_rep}"
                                )
                                nc.vector.tensor_scalar(
                                    es_i[:],
                                    ps[:].rearrange("p (h q) -> p h q", h=2),
                                    128.0 / 0.6931471805599453, 16248.5,
                                    mybir.AluOpType.mult, mybir.AluOpType.add,
                                )
                                es_rhs = lambda hh: es_i[:, hh, :].bitcast(BF16)
                            else:
                                es = expp.tile([P, 2, QT_W], BF16, tag="exp", name=f"es_r{_rep}")
                                nc.scalar.activation(
                                    es[:, :, off:QT_W],
                                    ps[:].rearrange("p (h q) -> p h q", h=2)[:, :, off:QT_W],
                                    mybir.ActivationFunctionType.Exp,
                                )
                                es_rhs = lambda hh: es[:, hh, off:QT_W]
                            if dj >= 0:
                                # only the first 128 valid columns straddle the
                                # diagonal; later columns have qq' >= 128 > kp
                                nc.vector.tensor_tensor(
                                    es[:, :, off : off + P],
                                    es[:, :, off : off + P],
                                    mask_sb[:],
                                    mybir.AluOpType.mult,
                                )
                            for hh in range(2):
                                h = hp * 2 + hh
                                nc.tensor.matmul(
                                    pc[0 : HD + 1, hh * QT_W + off : (hh + 1) * QT_W],
                                    v_t[kc][:, h, :],
                                    es_rhs(hh),
                                    start=(kc == 0),
                                    stop=(kc == nk - 1),
                                    skip_group_check=True,
                                )
                            if kc == 0 and pending:
                                pending.pop()()

                        def _normalize(pc=pc, ct=ct):
                            # copy unnormalized ctx + denominators to SBUF in
                            # one op; this frees the PSUM pair early so the
                            # next head pair's AV stream can start while the
                            # normalize tail runs entirely out of SBUF.
                            ctu = npool.tile(
                                [HD + 1, 2 * QT_W], BF16, tag="ctu", name=f"ctu_r{_rep}"
                            )
                            nc.vector.tensor_copy(ctu[:], pc[0 : HD + 1, :])
                            rec = npool.tile([1, 2 * QT_W], BF16, tag="rec", name=f"rec_r{_rep}")
                            with nc.allow_low_precision(reason="softmax denom recip"):
                                nc.vector.reciprocal(rec[:], ctu[HD : HD + 1, :])
                            pb = spsum.tile([P, 2 * QT_W], F32, tag="sc", name=f"pb_r{_rep}")
                            nc.tensor.matmul(
                                pb[0:HD, 0:QT_W], ones_hd[:], rec[:, 0:QT_W],
                                start=True, stop=True,
                            )
                            nc.tensor.matmul(
                                pb[0:HD, QT_W:], ones_hd[:], rec[:, QT_W:],
                                start=True, stop=True,
                            )
                            bc = npool.tile([HD, 2 * QT_W], BF16, tag="bc", name=f"bc_r{_rep}")
                            nc.vector.tensor_copy(bc[:], pb[0:HD, :])
                            nc.vector.tensor_tensor(
                                ct[0:HD, :], ctu[0:HD, 0:QT_W], bc[:, 0:QT_W],
                                mybir.AluOpType.mult,
                            )
                            nc.vector.tensor_tensor(
                                ct[HD:P, :], ctu[0:HD, QT_W:], bc[:, QT_W:],
                                mybir.AluOpType.mult,
                            )

                        pending.append(_normalize)

                def _outproj(qt):
                    # out projection for q tile qt (all head pairs done)
                    for qc4 in range(QT_W // P):
                        qc = qt * (QT_W // P) + qc4
                        ob = opool.tile([P, D], F32, tag="ob", name=f"ob_r{_rep}")
                        for ot in range(D // QT_W):
                            osl = slice(ot * QT_W, (ot + 1) * QT_W)
                            po = ppsum.tile([P, QT_W], F32, tag="pp", name=f"po_r{_rep}")
                            for c in range(NDT):
                                nc.tensor.matmul(
                                    po[:],
                                    ctxt[c, qt][:, qc4 * P : (qc4 + 1) * P],
                                    wo_t[c][:, osl],
                                    start=(c == 0),
                                    stop=(c == NDT - 1),
                                )
                            nc.vector.tensor_tensor(
                                ob[:, osl], po[:], bo_bc[:, osl], mybir.AluOpType.add
                            )
                        nc.sync.dma_start(out_d[qc * P : (qc + 1) * P, :], ob[:])

                # ---- interleaved emission. proj(st+1) is emitted BEFORE
                # outproj(st) so the shared proj-PSUM pool's slots recycle
                # through fast consumers only, and the PE priority heap can
                # fill exp-wait bubbles in attn(qt) with proj(st+1) work. ----
                def _proj(st):
                    for sc in range(4 * st, 4 * st + 4):
                        _proj_v(sc)
                    _proj_qk(st)

                pending = []
                _proj(0)
                if parts == "proj":
                    for st in range(1, NQT):
                        _proj(st)
                    nc.gpsimd.dma_start(out_d[0:P, 0:QT_W], qT[0, 0][:])
                else:
                    for st in range(NQT):
                        _attn(st, pending)
                        if st + 1 < NQT:
                            _proj(st + 1)
                        while pending:
                            pending.pop()()
                        if parts == "attn":
                            nc.gpsimd.dma_start(
                                out_d[st * P : (st + 1) * P, 0:QT_W], ctxt[0, st][:]
                            )
                    # all out-projections are emitted after the last q tile's
                    # attention: they are data-ready much earlier, so the PE
                    # priority heap uses them to fill exp-wait bubbles in the
                    # late attention tiles (which no projection work reaches).
                    if parts == "full":
                        for st in range(NQT):
                            _outproj(st)

    _split_waits(nc)
    return nc


def _mask():
    # mask[kp, (h, qq)] = 1 if kp <= qq else 0 (triangular causal, chunk-local,
    # materialized for both heads so the DVE multiply uses dense operands)
    kp = np.arange(P)[:, None]
    qq = np.arange(P)[None, :]
    m = (kp <= qq).astype(ml_dtypes.bfloat16)
    return np.concatenate([m, m], axis=1)


def kernel(x, Wq, bq, Wk, bk, Wv, bv, Wo, bo, _trace=False):
    x = np.asarray(x, np.float32)
    Wq, bq = np.asarray(Wq, np.float32), np.asarray(bq, np.float32)
    Wk, bk = np.asarray(Wk, np.float32), np.asarray(bk, np.float32)
    Wv, bv = np.asarray(Wv, np.float32), np.asarray(bv, np.float32)
    Wo, bo = np.asarray(Wo, np.float32), np.asarray(bo, np.float32)

    if "nc" not in _BUILT:
        _BUILT["nc"] = _build_kernel()
    nc = _BUILT["nc"]

    mask = _mask()
    zeros_bo = np.zeros_like(bo)
    xb = x.astype(ml_dtypes.bfloat16)
    Wqb = Wq.astype(ml_dtypes.bfloat16)
    Wkb = Wk.astype(ml_dtypes.bfloat16)
    Wvb = Wv.astype(ml_dtypes.bfloat16)
    Wob = Wo.astype(ml_dtypes.bfloat16)
    in_maps = []
    for c in range(8):
        b, hg = c // 2, c % 2
        cols = slice(hg * DL, (hg + 1) * DL)
        in_maps.append(
            {
                "x": np.ascontiguousarray(xb[b]),
                "wq": np.ascontiguousarray(Wqb[:, cols]),
                "wk": np.ascontiguousarray(Wkb[:, cols]),
                "wv": np.ascontiguousarray(Wvb[:, cols]),
                "wo": np.ascontiguousarray(Wob[cols, :]),
                "bq": np.ascontiguousarray(bq[cols].reshape(NDT, P).T),
                "bk": np.ascontiguousarray(bk[cols].reshape(NDT, P).T),
                "bv": np.ascontiguousarray(np.broadcast_to(bv[cols], (P, DL))),
                "bo": np.ascontiguousarray(
                    np.broadcast_to(bo if hg == 0 else zeros_bo, (P, D))
                ),
                "mask": mask,
            }
        )

    res = bass_utils.run_bass_kernel_spmd(
        nc, in_maps, core_ids=list(range(8)), trace=_trace
    )
    parts = [r["out"] for r in res.results]
    out = np.empty((B, S, D), np.float32)
    for b in range(B):
        out[b] = parts[2 * b] + parts[2 * b + 1]
    if _trace:
        return out, res
    return out

